# revision 1
# baseline (speedup 1.0000x reference)
"""Trainium2 Bass kernel for nn_MoEClassifier (moe_routing).

Model (per sample):
  x[16,5] -> flat 80 -> fc1(80->64) gelu -> fc2(64->64) gelu -> LN -> h
  u = user_table[user_id]  (16)
  gate: g_e = sum_r (h @ gU[e])_r * (u @ gV[e])_r + gb_e ; top-2 softmax -> w
  experts (dense): z_e = gelu(h @ e_w1[e] + e_b1[e]); LN(z); lpe = z @ e_w2[e] + e_b2
  logits = sum_e w_e * lpe_e   (10 classes)

Strategy: pure data-parallel across 8 NeuronCores (batch 131072 -> 16384/core).
On-chip layout is feature-major ([feature partitions, batch free]).  Per-sample
scalar math (LN rsqrt, top-2 gate) runs batch-major via PE transposes.
Expert LN is folded algebraically into the expert fc2 / combine stage:
  lpe = rs*( (z*g)@w2 - mu*(g@w2) ) + (beta@w2 + b2)
  logits = sum_e ws_e*A_e - sum_e wsm_e*gw2[e] + sum_e w_e*const[e]
with ws = w*rs, wsm = w*rs*mu.
"""
import sys, os

for _p in ("/opt/trn_rl_repo",):
    if _p not in sys.path:
        sys.path.insert(0, _p)

import numpy as np
from contextlib import ExitStack

import concourse.bass as bass
import concourse.tile as tile
from concourse import bacc, mybir

F32 = mybir.dt.float32
F32R = mybir.dt.float32r
I16 = mybir.dt.int16
I32 = mybir.dt.int32
AF = mybir.ActivationFunctionType
ALU = mybir.AluOpType

# Model dims (hardcoded per problem spec)
B = 131072
NCORES = 8
B_CORE = B // NCORES
IN_F = 80
EMB = 64
UDIM = 16
E = 16
RANK = 8
NCLS = 10
NUSERS = 1000
EPS_LN = 1e-5
TN = 512          # streaming tile width (one PSUM bank of fp32)
NCH = TN // 128   # 128-chunks per tile

# expert row order in the per-sample scalar block (see mu/m2 copy layout)
PERM = list(range(16))  # natural order (stats extraction preserves it)

MMDT_DEFAULT = "f32"   # "f32" (exact, 4 cyc/row) or "f32r" (~2e-4 rel; unreliable on HW here)


def _bc(ap, n):
    """broadcast the (size-1) innermost dim of an AP to n via stride 0"""
    return ap.to_broadcast(list(ap.shape[:-1]) + [n])


def build_program(b_core=B_CORE, mmdt=MMDT_DEFAULT, bufs=None):
    MMDT = F32R if mmdt == "f32r" else F32
    ntiles = b_core // TN
    bu = {"inp": 4, "work": 4, "scal": 4, "zsb": 9, "z2sb": 3, "osb": 4,
          "psm": 3, "psz": 2, "psf": 1, "psb2": 2}
    if bufs:
        bu.update(bufs)
    nc = bacc.Bacc("TRN2", target_bir_lowering=False, debug=False,
                   num_devices=NCORES)

    # ---------------- DRAM I/O ----------------
    d_x = nc.dram_tensor("x", [ntiles, IN_F, TN], MMDT, kind="ExternalInput")
    d_u = nc.dram_tensor("u", [ntiles, UDIM, TN], MMDT, kind="ExternalInput")
    d_out = nc.dram_tensor("out", [ntiles, NCH, 128, NCLS], F32, kind="ExternalOutput")

    def cin(name, shape, dt=F32):
        return nc.dram_tensor(name, shape, dt, kind="ExternalInput")

    d_ident = cin("ident", [128, 128])
    d_wbb1 = cin("wbb1", [IN_F, EMB], MMDT)
    d_wbb2 = cin("wbb2", [EMB, EMB], MMDT)
    d_b1 = cin("b1c", [EMB, 1])
    d_b2 = cin("b2c", [EMB, 1])
    d_beta = cin("betac", [EMB, 1])
    d_stat64 = cin("stat64", [128, 32])
    d_stl = cin("st_lhs", [2, 128], MMDT)
    d_wgU = cin("wgU", [EMB, 128], MMDT)
    d_wgV = cin("wgV", [UDIM, 128], MMDT)
    d_gsum = cin("gsum_lhs", [128, E], MMDT)
    d_gb = cin("gb_col", [E, 1])
    d_we1 = cin("we1", [128, 4, 128], MMDT)
    d_eb1 = cin("eb1", [128, 8])
    d_we2 = cin("we2", [128, 8, 32])
    d_wsb = cin("wsb_lhs", [48, 2, 128], MMDT)
    d_msum = cin("msum_lhs", [128, NCLS], MMDT)
    d_gw2c = cin("gw2c_lhs", [2 * E, NCLS], MMDT)

    with tile.TileContext(nc) as tc, ExitStack() as ctx:
        cpool = ctx.enter_context(tc.tile_pool(name="consts", bufs=1))
        p_in = ctx.enter_context(tc.tile_pool(name="inp", bufs=bu["inp"]))
        p_w = ctx.enter_context(tc.tile_pool(name="work", bufs=bu["work"]))
        p_sc = ctx.enter_context(tc.tile_pool(name="scal", bufs=bu["scal"]))
        p_z = ctx.enter_context(tc.tile_pool(name="zsb", bufs=bu["zsb"]))
        p_z2 = ctx.enter_context(tc.tile_pool(name="z2sb", bufs=bu["z2sb"]))
        p_out = ctx.enter_context(tc.tile_pool(name="osb", bufs=bu["osb"]))
        ps_m = ctx.enter_context(tc.tile_pool(name="psm", bufs=bu["psm"], space="PSUM"))
        ps_z = ctx.enter_context(tc.tile_pool(name="psz", bufs=bu["psz"], space="PSUM"))
        ps_f = ctx.enter_context(tc.tile_pool(name="psf", bufs=bu["psf"], space="PSUM"))
        ps_b = ctx.enter_context(tc.tile_pool(name="psb2", bufs=bu["psb2"], space="PSUM")) \
            if bu.get("psb2") else ps_m

        # ---------------- constants to SBUF ----------------
        c = {}
        for name, d, shape, dt in [
            ("ident", d_ident, [128, 128], F32),
            ("wbb1", d_wbb1, [IN_F, EMB], MMDT),
            ("wbb2", d_wbb2, [EMB, EMB], MMDT),
            ("b1", d_b1, [EMB, 1], F32),
            ("b2", d_b2, [EMB, 1], F32),
            ("beta", d_beta, [EMB, 1], F32),
            ("stat64", d_stat64, [128, 32], F32),
            ("stl", d_stl, [2, 128], MMDT),
            ("wgU", d_wgU, [EMB, 128], MMDT),
            ("wgV", d_wgV, [UDIM, 128], MMDT),
            ("gsum", d_gsum, [128, E], MMDT),
            ("gb", d_gb, [E, 1], F32),
            ("we1", d_we1, [128, 4, 128], MMDT),
            ("eb1", d_eb1, [128, 8], F32),
            ("we2", d_we2, [128, 8, 32], F32),
            ("wsb", d_wsb, [48, 2, 128], MMDT),
            ("msum", d_msum, [128, NCLS], MMDT),
            ("gw2c", d_gw2c, [2 * E, NCLS], MMDT),
        ]:
            t = cpool.tile(shape, dt, tag=name)
            nc.sync.dma_start(t[:], d.ap())
            c[name] = t

        ident = c["ident"]

        def tile_body(it):
            # ---------- load x / u (feature-major, host-prepped) ----------
            x_fm = p_in.tile([IN_F, TN], MMDT, tag="x_fm")
            nc.sync.dma_start(x_fm[:], d_x.ap()[it])
            u_fm = p_in.tile([UDIM, TN], MMDT, tag="u_fm")
            nc.sync.dma_start(u_fm[:], d_u.ap()[it])

            # ---------- backbone ----------
            ps1 = ps_m.tile([EMB, TN], F32, tag="psm")
            nc.tensor.matmul(ps1[:], c["wbb1"][:], x_fm[:], start=True, stop=True)
            h1 = p_w.tile([EMB, TN], MMDT, tag="h1")
            nc.scalar.activation(h1[:], ps1[:], AF.Gelu, bias=c["b1"][:])

            ps2 = ps_m.tile([EMB, TN], F32, tag="psm")
            nc.tensor.matmul(ps2[:], c["wbb2"][:], h1[:], start=True, stop=True)
            h2s = p_w.tile([128, TN], F32, tag="h2s")   # rows 0-63 h2, 64-127 h2^2
            nc.scalar.activation(h2s[0:EMB, :], ps2[:], AF.Gelu, bias=c["b2"][:])
            nc.scalar.activation(h2s[EMB:128, :], h2s[0:EMB, :], AF.Square)

            psb = ps_m.tile([2, TN], F32, tag="psm")     # mean(h2), mean(h2^2)
            nc.tensor.matmul(psb[:], c["stat64"][:, 0:2], h2s[:], start=True, stop=True)
            stats_bb = p_sc.tile([2, TN], F32, tag="stats_bb")
            nc.vector.tensor_copy(stats_bb[:], psb[:])

            # ---------- pass A: bb LN scalars (batch-major) ----------
            psA = ps_m.tile([128, NCH, 2], F32, tag="psm")
            for ch in range(NCH):
                nc.tensor.transpose(psA[:, ch, :], stats_bb[:, 128 * ch:128 * (ch + 1)],
                                    ident[0:2, 0:2])
            # var = (m2 + eps) - mu^2 ; rs = rsqrt(var) ; p = mu*rs
            sA = p_sc.tile([128, NCH, 2], F32, tag="sA")
            nc.vector.tensor_copy(sA[:], psA[:])
            tmpA = p_sc.tile([128, NCH], F32, tag="tmpA")
            nc.vector.tensor_tensor(tmpA[:], sA[:, :, 0], sA[:, :, 0], op=ALU.mult)
            vA = p_sc.tile([128, NCH], F32, tag="vA")
            nc.vector.scalar_tensor_tensor(vA[:], sA[:, :, 1], EPS_LN, tmpA[:],
                                           op0=ALU.add, op1=ALU.subtract)
            backA = p_sc.tile([128, NCH, 2], F32, tag="backA")
            rsA = backA[:, :, 0]
            _newton_rsqrt(nc, p_sc, vA[:], rsA, [128, NCH], "nA")
            nc.vector.tensor_tensor(backA[:, :, 1], rsA, sA[:, :, 0], op=ALU.mult)

            psBA = ps_m.tile([2, TN], F32, tag="psm")
            for ch in range(NCH):
                nc.tensor.transpose(psBA[:, 128 * ch:128 * (ch + 1)],
                                    backA[:, ch, :], ident[:])
            stf = p_sc.tile([2, TN], MMDT, tag="stf")
            nc.vector.tensor_copy(stf[:], psBA[:])

            # ---------- h = h2*S + (beta + T') ----------
            stp = ps_m.tile([128, TN], F32, tag="psm")
            nc.tensor.matmul(stp[:], c["stl"][:], stf[:], start=True, stop=True)
            tmph = p_w.tile([EMB, TN], F32, tag="tmph")
            nc.vector.tensor_tensor(tmph[:], h2s[0:EMB, :], stp[0:EMB, :], op=ALU.mult)
            h_fm = p_w.tile([128, TN], MMDT, tag="h_fm")
            nc.vector.scalar_tensor_tensor(h_fm[0:EMB, :], tmph[:], c["beta"][:],
                                           stp[EMB:128, :], op0=ALU.add, op1=ALU.add)
            nc.vector.tensor_copy(h_fm[EMB:128, :], h_fm[0:EMB, :])

            # ---------- gate ----------
            psU = ps_m.tile([128, TN], F32, tag="psm")
            nc.tensor.matmul(psU[:], c["wgU"][:], h_fm[0:EMB, :], start=True, stop=True)
            psV = ps_m.tile([128, TN], F32, tag="psm")
            nc.tensor.matmul(psV[:], c["wgV"][:], u_fm[:], start=True, stop=True)
            uVs = p_w.tile([128, TN], F32, tag="uVs")
            nc.scalar.copy(uVs[:], psV[:])
            gprod = p_w.tile([128, TN], MMDT, tag="gprod")
            nc.vector.tensor_tensor(gprod[:], psU[:], uVs[:], op=ALU.mult)
            psg = ps_m.tile([E, TN], F32, tag="psm")
            nc.tensor.matmul(psg[:], c["gsum"][:], gprod[:], start=True, stop=True)

            # ---------- experts fc1 (+gelu), z^2 ----------
            z_sb = []
            for q in range(4):
                zqA = ps_z.tile([128, TN], F32, tag=("psm" if bu.get("one_psum") else "zps"), name=f"zqA_{it}_{q}")
                zqB = ps_z.tile([128, TN], F32, tag=("psm" if bu.get("one_psum") else "zps"), name=f"zqB_{it}_{q}")
                nc.tensor.matmul(zqA[:], c["we1"][0:EMB, q, :], h_fm[0:EMB, :],
                                 start=True, stop=True, tile_position=(0, 0))
                nc.tensor.matmul(zqB[:], c["we1"][EMB:128, q, :],
                                 h_fm[EMB:128, :], start=True, stop=True,
                                 tile_position=(EMB, 0))
                for s, zq in enumerate((zqA, zqB)):
                    p = 2 * q + s
                    z = p_z.tile([128, TN], F32, tag="z_sb", name=f"z_{it}_{p}")
                    nc.scalar.activation(z[:], zq[:], AF.Gelu,
                                         bias=c["eb1"][:, p:p + 1])
                    z_sb.append(z)

            z2_sb = []
            for p in range(8):
                z2 = p_z2.tile([128, TN], F32, tag="z2_sb")
                eng = nc.gpsimd if p < 6 else nc.vector
                eng.tensor_tensor(z2[:], z_sb[p][:], z_sb[p][:], op=ALU.mult)
                z2_sb.append(z2)

            # ---------- expert stats (z^2) and fc2 (+mu), col-tiled ----------
            zst = [ps_m.tile([128, TN], F32, tag="psm", name=f"zst{it}_{i}") for i in range(2)]
            for grp in range(2):
                for j in range(4):
                    p = 4 * grp + j
                    nc.tensor.matmul(zst[grp][32 * j:32 * j + 32, :], c["stat64"][:],
                                     z2_sb[p][:], start=True, stop=True,
                                     tile_position=(0, 32 * j))
            fc2 = [ps_f.tile([128, TN], F32, tag=("psm" if bu.get("one_psum") else "fc2"), name=f"fc2_{it}_{i}") for i in range(2)]
            for grp in range(2):
                for j in range(4):
                    p = 4 * grp + j
                    nc.tensor.matmul(fc2[grp][32 * j:32 * j + 32, :],
                                     c["we2"][:, p, :], z_sb[p][:],
                                     start=True, stop=True, tile_position=(0, 32 * j))

            # ---------- stats to batch-major via full-bank transposes ----------
            # copy fc2 / zst psum banks to SBUF (fc2sb also feeds combine)
            fc2sb, zstsb = [], []
            for b in range(2):
                t = p_w.tile([128, TN], F32, tag="fc2sb", name=f"fc2sb_{it}_{b}")
                nc.scalar.copy(t[:], fc2[b][:])
                fc2sb.append(t)
                t2 = p_w.tile([128, TN], F32, tag="zstsb", name=f"zstsb_{it}_{b}")
                nc.scalar.copy(t2[:], zst[b][:])
                zstsb.append(t2)
            g_sb = p_sc.tile([E, TN], F32, tag="g_sb")
            nc.vector.tensor_scalar(g_sb[:], psg[:], c["gb"][:], None, op0=ALU.add)

            yield  # ---- frontend/backend pipeline split ----

            muB = p_sc.tile([128, NCH, E], F32, tag="muB")
            m2B = p_sc.tile([128, NCH, E], F32, tag="m2B")

            def _extract(src_ps, dst, base):
                sap = src_ps[:, :, 0]
                a = sap.ap
                sap2 = bass.AP(tensor=sap.tensor, offset=sap.offset + base,
                               ap=[a[0], a[1], [32, 4], [1, 2]])
                dap = dst.ap
                dst2 = bass.AP(tensor=dst.tensor, offset=dst.offset,
                               ap=[dap[0], dap[1], [2, 4], [1, 2]])
                nc.vector.tensor_copy(dst2, sap2)

            for b in range(2):
                psT = ps_b.tile([128, NCH, 128], F32, tag="psb2", name=f"psTf_{it}_{b}")
                for ch in range(NCH):
                    nc.tensor.transpose(psT[:, ch, :],
                                        fc2sb[b][:, 128 * ch:128 * (ch + 1)], ident[:])
                _extract(psT, muB[:, :, 8 * b:8 * b + 8], 20)
            for b in range(2):
                psT = ps_b.tile([128, NCH, 128], F32, tag="psb2", name=f"psTz_{it}_{b}")
                for ch in range(NCH):
                    nc.tensor.transpose(psT[:, ch, :],
                                        zstsb[b][:, 128 * ch:128 * (ch + 1)], ident[:])
                _extract(psT, m2B[:, :, 8 * b:8 * b + 8], 0)

            psTg = ps_b.tile([128, NCH, E], F32, tag="psb2", name=f"psTg_{it}")
            for ch in range(NCH):
                nc.tensor.transpose(psTg[:, ch, :], g_sb[:, 128 * ch:128 * (ch + 1)],
                                    ident[0:E, 0:E])
            gcp = p_sc.tile([128, NCH, E], F32, tag="gcp")
            nc.scalar.copy(gcp[:], psTg[:])

            # ---------- pass B math ----------
            tmpB = p_sc.tile([128, NCH, E], F32, tag="tmpB")
            nc.vector.tensor_tensor(tmpB[:], muB[:], muB[:], op=ALU.mult)
            vB = p_sc.tile([128, NCH, E], F32, tag="vB")
            nc.vector.scalar_tensor_tensor(vB[:], m2B[:], EPS_LN, tmpB[:],
                                           op0=ALU.add, op1=ALU.subtract)
            rsB = p_sc.tile([128, NCH, E], F32, tag="rsB")
            _newton_rsqrt(nc, p_sc, vB[:], rsB[:], [128, NCH, E], "nB")
            vm8 = p_sc.tile([128, NCH, 8], F32, tag="vm8")
            for ch in range(NCH):
                nc.vector.max(vm8[:, ch, :], gcp[:, ch, :])
            dg = p_sc.tile([128, NCH], F32, tag="dg")
            nc.vector.tensor_tensor(dg[:], vm8[:, :, 0], vm8[:, :, 1], op=ALU.subtract)
            th = p_sc.tile([128, NCH], F32, tag="th")
            nc.scalar.activation(th[:], dg[:], AF.Tanh, scale=0.5)
            w12 = p_sc.tile([128, NCH, 2], F32, tag="w12")
            nc.vector.tensor_scalar(w12[:, :, 0], th[:], 0.5, 0.5, op0=ALU.mult, op1=ALU.add)
            nc.vector.tensor_scalar(w12[:, :, 1], th[:], -0.5, 0.5, op0=ALU.mult, op1=ALU.add)

            is1 = p_sc.tile([128, NCH, E], F32, tag="is1")
            nc.vector.tensor_tensor(is1[:], gcp[:], _bc(vm8[:, :, 0:1], E), op=ALU.is_equal)
            is2 = p_sc.tile([128, NCH, E], F32, tag="is2")
            nc.vector.tensor_tensor(is2[:], gcp[:], _bc(vm8[:, :, 1:2], E), op=ALU.is_equal)
            w1t = p_sc.tile([128, NCH, E], F32, tag="w1t")
            nc.vector.tensor_tensor(w1t[:], is1[:], _bc(w12[:, :, 0:1], E), op=ALU.mult)
            w2t = p_sc.tile([128, NCH, E], F32, tag="w2t")
            nc.vector.tensor_tensor(w2t[:], is2[:], _bc(w12[:, :, 1:2], E), op=ALU.mult)

            # back block: cols 0-15 wsm, 16-31 w, 32-47 ws, 48-63 pad
            backB = p_sc.tile([128, NCH, 64], F32, tag="backB")
            nc.gpsimd.memset(backB[:].rearrange("p c k -> p (c k)"), 0.0)
            nc.vector.tensor_tensor(backB[:, :, 16:32], w1t[:], w2t[:], op=ALU.add)
            nc.vector.tensor_tensor(backB[:, :, 32:48], backB[:, :, 16:32], rsB[:], op=ALU.mult)
            nc.vector.tensor_tensor(backB[:, :, 0:16], backB[:, :, 32:48], muB[:],
                                    op=ALU.mult)

            # 2 transposes of [128,128] (chunk-pairs, 64-padded); cf de-interleaves
            psBB = ps_b.tile([128, 2, 128], F32, tag="psb2")
            backBv = backB[:].rearrange("p c k -> p (c k)")
            for hh in range(2):
                nc.tensor.transpose(psBB[:, hh, :],
                                    backBv[:, 128 * hh:128 * (hh + 1)], ident[:])
            cf = p_sc.tile([48, TN], MMDT, tag="cf")
            cfv = cf[:].rearrange("p (h c q) -> p h c q", h=2, c=2, q=128)
            nc.vector.tensor_copy(cfv[:, :, 0, :], psBB[0:48, :, :])
            nc.vector.tensor_copy(cfv[:, :, 1, :], psBB[64:112, :, :])

            # ---------- combine ----------
            lg = ps_b.tile([NCLS, TN], F32, tag="psb2")
            prods = []
            for b in range(2):
                wsr = ps_b.tile([128, TN], F32, tag="psb2")
                nc.tensor.matmul(wsr[:], c["wsb"][32:48, b, :], cf[32:48, :],
                                 start=True, stop=True)
                prod = p_w.tile([128, TN], F32, tag="prod", name=f"prod_{it}_{b}")
                nc.vector.tensor_tensor(prod[:], fc2sb[b][:], wsr[:], op=ALU.mult)
                prods.append(prod)
            psum2 = p_w.tile([128, TN], MMDT, tag="psum2")
            nc.vector.tensor_tensor(psum2[:], prods[0][:], prods[1][:], op=ALU.add)
            nc.tensor.matmul(lg[:], c["msum"][:], psum2[:], start=True, stop=False)
            nc.tensor.matmul(lg[:], c["gw2c"][:], cf[0:32, :], start=False, stop=True)

            lsb = p_out.tile([NCLS, TN], F32, tag="lsb")
            nc.scalar.copy(lsb[:], lg[:])
            psL = ps_b.tile([128, NCH * NCLS], F32, tag="psb2")
            for ch in range(NCH):
                nc.tensor.transpose(psL[:, NCLS * ch:NCLS * (ch + 1)],
                                    lsb[:, 128 * ch:128 * (ch + 1)],
                                    ident[0:NCLS, 0:NCLS])
            osb = p_out.tile([128, NCH, NCLS], F32, tag="osb")
            nc.vector.tensor_copy(osb[:], psL[:])
            nc.sync.dma_start(d_out.ap()[it].rearrange("c p k -> p c k"), osb[:])

        SKEW = int(os.environ.get("KSKEW", "1"))
        gens = []
        for it in range(ntiles):
            gen = tile_body(it)
            next(gen)
            gens.append(gen)
            if it >= SKEW:
                for _ in gens[it - SKEW]:
                    pass
        for it in range(max(0, ntiles - SKEW), ntiles):
            for _ in gens[it]:
                pass

    nc.compile()
    return nc


def _newton_rsqrt(nc, pool, v_ap, out_ap, shape, tag, eng=None):
    """out = 1/sqrt(v) via quake seed + Newton iterations."""
    eng = eng or nc.vector
    r = pool.tile(shape, F32, tag=tag + "_r")
    t = pool.tile(shape, F32, tag=tag + "_t")
    eng.tensor_scalar(r[:].bitcast(I32), v_ap.bitcast(I32), 1, None,
                      op0=ALU.logical_shift_right)
    eng.tensor_scalar(r[:].bitcast(I32), r[:].bitcast(I32), -1, 0x5F3759DF,
                      op0=ALU.mult, op1=ALU.add)
    niter = int(os.environ.get("KNEWTON", "2"))
    for i in range(niter):
        dst = out_ap if i == niter - 1 else r[:]
        eng.tensor_tensor(t[:], r[:], r[:], op=ALU.mult)
        eng.tensor_tensor(t[:], t[:], v_ap, op=ALU.mult)
        eng.tensor_scalar(t[:], t[:], -0.5, 1.5, op0=ALU.mult, op1=ALU.add)
        eng.tensor_tensor(dst, r[:], t[:], op=ALU.mult)


# ---------------------------------------------------------------------------
# host-side weight prep
# ---------------------------------------------------------------------------
def prep_consts(inp):
    f = np.float32
    gU, gV, gb = inp["gU"].astype(f), inp["gV"].astype(f), inp["gb"].astype(f)
    e_w1, e_b1 = inp["e_w1"].astype(f), inp["e_b1"].astype(f)
    e_g, e_beta = inp["e_g"].astype(f), inp["e_beta"].astype(f)
    e_w2, e_b2 = inp["e_w2"].astype(f), inp["e_b2"].astype(f)
    ut = inp["ut"].astype(f)
    bb_g, bb_beta = inp["bb_g"].astype(f), inp["bb_beta"].astype(f)

    cns = {}
    cns["ident"] = np.eye(128, dtype=f)
    cns["wbb1"] = inp["bb_w1"].astype(f)
    cns["wbb2"] = inp["bb_w2"].astype(f)
    cns["b1c"] = inp["bb_b1"].astype(f).reshape(EMB, 1)
    cns["b2c"] = inp["bb_b2"].astype(f).reshape(EMB, 1)
    cns["betac"] = bb_beta.reshape(EMB, 1)

    st = np.zeros((128, 32), f)
    st[0:64, 0] = 1.0 / 64
    st[64:128, 1] = 1.0 / 64
    cns["stat64"] = st

    stl = np.zeros((2, 128), f)
    stl[0, 0:64] = bb_g
    stl[1, 64:128] = -bb_g
    cns["st_lhs"] = stl

    wgU = np.zeros((EMB, 128), f)
    wgV = np.zeros((UDIM, 128), f)
    for e in range(E):
        wgU[:, e * RANK:(e + 1) * RANK] = gU[e]
        wgV[:, e * RANK:(e + 1) * RANK] = gV[e]
    cns["wgU"] = wgU
    cns["wgV"] = wgV

    gs = np.zeros((128, E), f)
    for i, e in enumerate(PERM):
        gs[e * RANK:(e + 1) * RANK, i] = 1.0
    cns["gsum_lhs"] = gs
    cns["gb_col"] = gb[PERM].reshape(E, 1)

    we1 = np.zeros((128, 4, 128), f)
    eb1 = np.zeros((128, 8), f)
    for q in range(4):
        # row-tile A (partitions 0-63) computes pair 2q, tile B pair 2q+1
        we1[0:64, q, :] = np.concatenate([e_w1[4 * q], e_w1[4 * q + 1]], axis=1)
        we1[64:128, q, :] = np.concatenate([e_w1[4 * q + 2], e_w1[4 * q + 3]], axis=1)
    for p in range(8):
        eb1[0:64, p] = e_b1[2 * p]
        eb1[64:128, p] = e_b1[2 * p + 1]
    cns["we1"] = we1
    cns["eb1"] = eb1

    we2 = np.zeros((128, 8, 32), f)
    for p in range(8):
        e0, e1 = 2 * p, 2 * p + 1
        we2[0:64, p, 0:10] = e_g[e0][:, None] * e_w2[e0]
        we2[64:128, p, 10:20] = e_g[e1][:, None] * e_w2[e1]
        we2[0:64, p, 20] = 1.0 / 64
        we2[64:128, p, 21] = 1.0 / 64
    cns["we2"] = we2

    wsb = np.zeros((48, 2, 128), f)
    for i, e in enumerate(PERM):
        p, q = e // 2, e % 2
        b, j = p // 4, p % 4
        wsb[32 + i, b, 32 * j + 10 * q:32 * j + 10 * q + 10] = 1.0
    cns["wsb_lhs"] = wsb

    ms = np.zeros((128, NCLS), f)
    for j in range(4):
        for q in range(2):
            for cc in range(NCLS):
                ms[32 * j + 10 * q + cc, cc] = 1.0
    cns["msum_lhs"] = ms

    gw2 = np.einsum("ed,edc->ec", e_g, e_w2)
    cst = np.einsum("ed,edc->ec", e_beta, e_w2) + e_b2
    gw2c = np.zeros((2 * E, NCLS), f)
    gw2c[0:E] = -gw2[PERM]
    gw2c[E:2 * E] = cst[PERM]
    cns["gw2c_lhs"] = gw2c

    return cns


def shard_inputs(x, user_ids, ut, b_core):
    """x [B,80] -> per-core [nt,80,512] feature-major; u gathered+transposed."""
    ncores = x.shape[0] // b_core
    nt = b_core // TN
    xs = np.ascontiguousarray(
        x.reshape(ncores, nt, TN, IN_F).transpose(0, 1, 3, 2).astype(np.float32))
    u = ut.astype(np.float32)[user_ids]          # [B, 16]
    us = np.ascontiguousarray(
        u.reshape(ncores, nt, TN, UDIM).transpose(0, 1, 3, 2))
    return xs, us


_CACHE = {}


def _get_program(b_core, mmdt):
    key = (b_core, mmdt)
    if key not in _CACHE:
        _CACHE[key] = build_program(b_core, mmdt)
    return _CACHE[key]


def kernel(**inputs):
    from concourse.bass_utils import run_bass_kernel_spmd
    mmdt = os.environ.get("KMMDT", MMDT_DEFAULT)
    x = np.asarray(inputs["x"], np.float32).reshape(B, IN_F)
    uids = np.asarray(inputs["user_ids"]).astype(np.int64)
    nc = _get_program(B_CORE, mmdt)
    cns = prep_consts({k: np.asarray(v) for k, v in inputs.items()})
    xs, us = shard_inputs(x, uids, np.asarray(inputs["ut"]), B_CORE)
    in_maps = []
    for k in range(NCORES):
        m = dict(cns)
        m["x"] = xs[k]
        m["u"] = us[k]
        in_maps.append(m)
    res = run_bass_kernel_spmd(nc, in_maps, core_ids=list(range(NCORES)))
    out = np.concatenate([r["out"].reshape(B_CORE, NCLS) for r in res.results], axis=0)
    return out.astype(np.float32)



# revision 55
# speedup vs baseline: 3.2183x; 3.2183x over previous
"""Trainium2 Bass kernel for nn_MoEClassifier (moe_routing).

Model (per sample):
  x[16,5] -> flat 80 -> fc1(80->64) gelu -> fc2(64->64) gelu -> LN -> h
  u = user_table[user_id]  (16)
  gate: g_e = sum_r (h @ gU[e])_r * (u @ gV[e])_r + gb_e ; top-2 softmax -> w
  experts (dense): z_e = gelu(h @ e_w1[e] + e_b1[e]); LN(z); lpe = z @ e_w2[e] + e_b2
  logits = sum_e w_e * lpe_e   (10 classes)

Strategy: pure data-parallel across 8 NeuronCores (batch 131072 -> 16384/core).
Feature-major on-chip layout ([feature partitions, batch free]).

Precision split (the gate's top-2 selection is tie-sensitive: min |g2-g3| gap
on this input set is ~3e-7, and bf16-level gate noise flips >1000 samples):
  - f32 path: backbone fc1/fc2, bb-LN stats + Newton rsqrt, gate A-matmul.
  - bf16 path (4x cheaper matmuls in cycles/row): everything expert-side
    (fc1/fc2/LN-stats/combine) plus the h broadcast used only by experts.

Gate refactor: h = LN(h2)*g+beta folds through the bilinear gate as
  g_e = rs * A_e - (mu*rs) * B_e + D_e
  A_e = sum_r psU0_er * uV_er      (device: psU0 = (g.wgU)^T @ h2, f32)
  B[u,e] = sum_r (wgU^T g)_er uV[u,er],  D[u,e] = sum_r (wgU^T beta)_er uV[u,er] + gb_e
(B, D, uV are host-precomputed per-user tables — weight prep, like prep_consts.)

Expert-LN fold (as baseline): lpe = rs*((z*g)@w2 - mu*(g@w2)) + (beta@w2+b2);
logits = sum_e ws_e*A'_e - sum_e wsm_e*gw2_e + sum_e w_e*cst_e, ws=w*rs, wsm=w*rs*mu.
Expert mu AND m2=mean(z^2) stats ride in the fc2 PSUM banks: block rows
32j+{0,1} = mu (via extra we2 columns), rows 32j+{2,3} = m2 (2nd matmul of an
accumulation group contracting z^2); cls at rows 32j+4..24.  Stats go
batch-major via full-bank bf16 transposes + one strided extraction copy.

fp16 hi/lo split matmuls (1 cyc/row vs 4 for f32, ~2^-22 accurate, verified
0 top-2 flips on this input set): bb fc1 (x split on host), bb-LN stats and
the gate A-matmul (h2 split on device).

Execution: 14-phase software pipeline, one phase per tile per outer step,
oldest tile first, so every cross-engine dependency crosses a step boundary
and each engine's FIFO receives work in data-ready order.  PSUM: 8 banks =
2(bb) + 2(z) + 2(fc) + 2(shared small).
"""
import sys, os

for _p in ("/opt/trn_rl_repo",):
    if _p not in sys.path:
        sys.path.insert(0, _p)

import numpy as np
from contextlib import ExitStack

import concourse.bass as bass
import concourse.tile as tile
from concourse import bacc, mybir

F32 = mybir.dt.float32
BF16 = mybir.dt.bfloat16
FP16 = mybir.dt.float16
I32 = mybir.dt.int32
AF = mybir.ActivationFunctionType
ALU = mybir.AluOpType

B = 131072
NCORES = 8
B_CORE = B // NCORES
IN_F = 80
EMB = 64
UDIM = 16
E = 16
RANK = 8
NCLS = 10
NUSERS = 1000
EPS_LN = 1e-5
TN = 512
NCH = TN // 128


def _bc(ap, n):
    """broadcast the (size-1) innermost dim of an AP to n via stride 0"""
    return ap.to_broadcast(list(ap.shape[:-1]) + [n])


def _pslice(t, base, cnt):
    """partition slice [base:base+cnt] of tile t's full AP"""
    return t[base:base + cnt]



# packed constant layouts: name -> (partitions, col offset, col width)
CF32_OFF = {
    "identf": (128, 0, 128), "wbb1": (IN_F, 128, EMB), "wbb2": (EMB, 192, EMB),
    "b1": (EMB, 256, 1), "b2": (EMB, 257, 1), "beta": (EMB, 258, 1),
    "stat2": (128, 259, 2), "wgU0": (EMB, 261, 128), "gsum": (128, 389, E),
    "eb1": (128, 405, 8),
}
CF32_COLS = 413
CB16_OFF = {
    "identb": (128, 0, 128), "stlb": (2, 128, 128), "we1b": (EMB, 256, 1024),
    "we2b": (128, 1280, 256), "zwb": (128, 1536, 32), "wsbb": (48, 1568, 256),
    "msumb": (128, 1824, NCLS), "gw2cb": (2 * E, 1834, NCLS),
    "permg": (128, 1844, 32),
}
CB16_COLS = 1876


def build_program(b_core=B_CORE, mmdt="hybrid", bufs=None):
    ntiles = b_core // TN
    nc = bacc.Bacc("TRN2", target_bir_lowering=False, debug=False,
                   num_devices=NCORES)

    # ---------------- DRAM I/O ----------------
    d_x = nc.dram_tensor("x", [ntiles, IN_F, 2 * TN], FP16, kind="ExternalInput")
    d_u = nc.dram_tensor("u", [ntiles, 128, TN], F32, kind="ExternalInput")
    d_BD = nc.dram_tensor("BD", [ntiles, 128, NCH, 2 * E], F32, kind="ExternalInput")
    d_out = nc.dram_tensor("out", [ntiles, NCH, 128, NCLS], F32, kind="ExternalOutput")

    def cin(name, shape, dt=F32):
        return nc.dram_tensor(name, shape, dt, kind="ExternalInput")

    d_cf32 = cin("cf32", [128, CF32_COLS])
    d_cb16 = cin("cb16", [128, CB16_COLS], BF16)
    d_ch16 = cin("ch16", [128, 388], FP16)   # wbb1 h/l | wgU0 h/l | stat2 | zeros2

    bu = {"inp": 3, "work": 3, "scal": 3, "zsb": 9, "z2sb": 3, "osb": 3,
          "psbb": 2, "psz": 2, "psfc": 2, "pssm": 2}
    for k in list(bu):
        v = os.environ.get("KB_" + k)
        if v:
            bu[k] = int(v)
    if bufs:
        bu.update(bufs)

    with tile.TileContext(nc) as tc, ExitStack() as ctx:
        cpool = ctx.enter_context(tc.tile_pool(name="consts", bufs=1))
        p_in = ctx.enter_context(tc.tile_pool(name="inp", bufs=bu["inp"]))
        p_w = ctx.enter_context(tc.tile_pool(name="work", bufs=bu["work"]))
        p_sc = ctx.enter_context(tc.tile_pool(name="scal", bufs=bu["scal"]))
        p_z = ctx.enter_context(tc.tile_pool(name="zsb", bufs=bu["zsb"]))
        p_z2 = ctx.enter_context(tc.tile_pool(name="z2sb", bufs=bu["z2sb"]))
        p_out = ctx.enter_context(tc.tile_pool(name="osb", bufs=bu["osb"]))
        ps_bb = ctx.enter_context(tc.tile_pool(name="psbb", bufs=bu["psbb"], space="PSUM"))
        ps_z = ctx.enter_context(tc.tile_pool(name="psz", bufs=bu["psz"], space="PSUM"))
        ps_fc = ctx.enter_context(tc.tile_pool(name="psfc", bufs=bu["psfc"], space="PSUM"))
        ps_sm = ctx.enter_context(tc.tile_pool(name="pssm", bufs=bu["pssm"], space="PSUM"))

        # ------------- constants: two packed DMAs, sliced views -------------
        t32 = cpool.tile([128, CF32_COLS], F32, tag="cf32", name="c_f32")
        nc.sync.dma_start(t32[:], d_cf32.ap())
        t16 = cpool.tile([128, CB16_COLS], BF16, tag="cb16", name="c_b16")
        nc.sync.dma_start(t16[:], d_cb16.ap())
        c = {}
        for name, (p, o, w) in CF32_OFF.items():
            c[name] = t32[0:p, o:o + w]
        for name, (p, o, w) in CB16_OFF.items():
            c[name] = t16[0:p, o:o + w]
        t_h16 = cpool.tile([128, 388], FP16, tag="ch16", name="c_h16")
        nc.sync.dma_start(t_h16[:], d_ch16.ap())
        c["wbb1h"] = t_h16[0:IN_F, 0:EMB]
        c["wbb1l"] = t_h16[0:IN_F, EMB:2 * EMB]
        c["wgU0h"] = t_h16[0:EMB, 128:256]
        c["wgU0l"] = t_h16[0:EMB, 256:384]
        c["stat2h"] = t_h16[0:128, 384:386]
        c["zc2"] = t_h16[0:128, 386:388]
        c["we1b"] = c["we1b"].rearrange("p (a b) -> p a b", a=8, b=128)
        c["we2b"] = c["we2b"].rearrange("p (a b) -> p a b", a=8, b=32)
        c["wsbb"] = c["wsbb"].rearrange("p (a b) -> p a b", a=2, b=128)
        identf = c["identf"]
        identb = c["identb"]

        def tile_body(it):
            # ==== P0: input DMAs + backbone fc1 ====
            x_fm = p_in.tile([IN_F, 2 * TN], FP16, tag="x_fm", bufs=3, name=f"x_{it}")
            nc.sync.dma_start(x_fm[:], d_x.ap()[it])
            u_fm = p_in.tile([128, TN], F32, tag="u_fm", bufs=4, name=f"u_{it}")
            nc.sync.dma_start(u_fm[:], d_u.ap()[it])
            BD_t = p_in.tile([128, NCH, 2 * E], F32, tag="BD", bufs=7, name=f"BD_{it}")
            nc.sync.dma_start(BD_t[:], d_BD.ap()[it])
            BT_t = BD_t[:, :, 0:E]
            DT_t = BD_t[:, :, E:2 * E]

            ps1 = ps_bb.tile([EMB, TN], F32, tag="bb", name=f"ps1_{it}")
            nc.tensor.matmul(ps1[:], c["wbb1h"], x_fm[:, 0:TN], start=True, stop=False)
            nc.tensor.matmul(ps1[:], c["wbb1h"], x_fm[:, TN:2 * TN], start=False, stop=False)
            nc.tensor.matmul(ps1[:], c["wbb1l"], x_fm[:, 0:TN], start=False, stop=True)
            h1 = p_w.tile([EMB, TN], F32, tag="h1", bufs=3, name=f"h1_{it}")
            nc.scalar.activation(h1[:], ps1[:], AF.Gelu, bias=c["b1"])

            yield  # ==== P1: backbone fc2 + square ====
            ps2 = ps_bb.tile([EMB, TN], F32, tag="bb", name=f"ps2_{it}")
            nc.tensor.matmul(ps2[:], c["wbb2"], h1[:], start=True, stop=True)
            h2s = p_w.tile([128, TN], F32, tag="h2s", bufs=6, name=f"h2s_{it}")
            nc.scalar.activation(h2s[0:EMB, :], ps2[:], AF.Gelu, bias=c["b2"])
            nc.scalar.activation(h2s[EMB:128, :], h2s[0:EMB, :], AF.Square)
            h2hi = p_w.tile([128, TN], FP16, tag="h2hi", bufs=3, name=f"h2hi_{it}")
            nc.gpsimd.tensor_tensor(h2hi[:], h2s[:], _bc(c["zc2"][:, 0:1], TN),
                                    op=ALU.add)
            h2lo = p_w.tile([128, TN], FP16, tag="h2lo", bufs=3, name=f"h2lo_{it}")
            nc.gpsimd.tensor_tensor(h2lo[:], h2s[:], h2hi[:], op=ALU.subtract)

            yield  # ==== P2: bb-LN stats + gate A matmul ====
            psb = ps_bb.tile([2, TN], F32, tag="bb", name=f"psb_{it}")
            nc.tensor.matmul(psb[:], c["stat2h"], h2hi[:], start=True, stop=False)
            nc.tensor.matmul(psb[:], c["stat2h"], h2lo[:], start=False, stop=True)
            stats_bb = p_sc.tile([2, TN], F32, tag="stats_bb", bufs=3, name=f"sbb_{it}")
            nc.scalar.copy(stats_bb[:], psb[:])
            psU0 = ps_bb.tile([128, TN], F32, tag="bb", name=f"psU0_{it}")
            nc.tensor.matmul(psU0[:], c["wgU0h"], h2hi[0:EMB, :], start=True, stop=False)
            nc.tensor.matmul(psU0[:], c["wgU0h"], h2lo[0:EMB, :], start=False, stop=False)
            nc.tensor.matmul(psU0[:], c["wgU0l"], h2hi[0:EMB, :], start=False, stop=True)
            gprod = p_w.tile([128, TN], F32, tag="gprod", bufs=4, name=f"gprod_{it}")
            nc.vector.tensor_tensor(gprod[:], psU0[:], u_fm[:], op=ALU.mult)

            yield  # ==== P3: pass A (bb LN scalars) + gate sum ====
            psA = ps_sm.tile([128, NCH, 2], F32, tag="sm", name=f"psA_{it}")
            for ch in range(NCH):
                nc.tensor.transpose(psA[:, ch, :], stats_bb[:, 128 * ch:128 * (ch + 1)],
                                    identf[0:2, 0:2])
            sA = p_sc.tile([128, NCH, 2], F32, tag="sA", bufs=3, name=f"sA_{it}")
            nc.scalar.copy(sA[:], psA[:])
            tmpA = p_sc.tile([128, NCH], F32, tag="tmpA", bufs=3, name=f"tmpA_{it}")
            nc.vector.tensor_tensor(tmpA[:], sA[:, :, 0], sA[:, :, 0], op=ALU.mult)
            vA = p_sc.tile([128, NCH], F32, tag="vA", bufs=3, name=f"vA_{it}")
            nc.vector.scalar_tensor_tensor(vA[:], sA[:, :, 1], EPS_LN, tmpA[:],
                                           op0=ALU.add, op1=ALU.subtract)
            backA = p_sc.tile([128, NCH, 2], F32, tag="backA", bufs=4, name=f"backA_{it}")
            rsA = backA[:, :, 0]
            _newton_rsqrt(nc, p_sc, vA[:], rsA, [128, NCH], f"nA_{it}", niter=2)
            nc.vector.tensor_tensor(backA[:, :, 1], rsA, sA[:, :, 0], op=ALU.mult)
            backAb = p_sc.tile([128, NCH, 2], BF16, tag="backAb", bufs=3, name=f"backAb_{it}")
            nc.vector.tensor_copy(backAb[:], backA[:])

            psg = ps_sm.tile([E, TN], F32, tag="sm", name=f"psg_{it}")
            nc.tensor.matmul(psg[:], c["gsum"], gprod[:], start=True, stop=True)
            A_sb = p_sc.tile([E, TN], F32, tag="A_sb", bufs=4, name=f"Asb_{it}")
            nc.scalar.copy(A_sb[:], psg[:])

            yield  # ==== P4: rs/p broadcast transpose ====
            psBA = ps_sm.tile([2, TN], BF16, tag="sm", name=f"psBA_{it}")
            for ch in range(NCH):
                nc.tensor.transpose(psBA[:, 128 * ch:128 * (ch + 1)],
                                    backAb[:, ch, :], identb)
            stf = p_sc.tile([2, TN], BF16, tag="stf", bufs=3, name=f"stf_{it}")
            nc.scalar.copy(stf[:], psBA[:])

            yield  # ==== P5: h for experts + batch-major gate ====
            stp = ps_sm.tile([128, TN], F32, tag="sm", name=f"stp_{it}")
            nc.tensor.matmul(stp[:], c["stlb"], stf[:], start=True, stop=True)
            t1h = p_w.tile([EMB, TN], BF16, tag="t1h", bufs=3, name=f"t1h_{it}")
            nc.vector.tensor_tensor(t1h[:], h2s[0:EMB, :], stp[0:EMB, :], op=ALU.mult)
            hb = p_w.tile([EMB, TN], BF16, tag="hb", bufs=4, name=f"hb_{it}")
            nc.vector.scalar_tensor_tensor(hb[:], t1h[:], c["beta"],
                                           stp[EMB:128, :], op0=ALU.add, op1=ALU.add)

            psAT = ps_sm.tile([128, NCH, E], F32, tag="sm", name=f"psAT_{it}")
            for ch in range(NCH):
                nc.tensor.transpose(psAT[:, ch, :], A_sb[:, 128 * ch:128 * (ch + 1)],
                                    identf[0:E, 0:E])
            ATc = p_sc.tile([128, NCH, E], F32, tag="ATc", bufs=3, name=f"ATc_{it}")
            nc.scalar.copy(ATc[:], psAT[:])
            # g = rs*A - p*B + D    (batch-major, f32)
            g1t = p_sc.tile([128, NCH, E], F32, tag="g1t", bufs=3, name=f"g1t_{it}")
            nc.vector.tensor_tensor(g1t[:], ATc[:], _bc(backA[:, :, 0:1], E), op=ALU.mult)
            g2t = p_sc.tile([128, NCH, E], F32, tag="g2t", bufs=3, name=f"g2t_{it}")
            nc.vector.tensor_tensor(g2t[:], BT_t, _bc(backA[:, :, 1:2], E), op=ALU.mult)
            g3t = p_sc.tile([128, NCH, E], F32, tag="g3t", bufs=3, name=f"g3t_{it}")
            nc.vector.tensor_tensor(g3t[:], g1t[:], g2t[:], op=ALU.subtract)
            gcp = p_sc.tile([128, NCH, E], F32, tag="gcp", bufs=6, name=f"gcp_{it}")
            nc.vector.tensor_tensor(gcp[:], g3t[:], DT_t, op=ALU.add)

            yield  # ==== P6: experts fc1 + gelu + z^2 ====
            z_sb = []
            for p in range(8):
                zq = ps_z.tile([128, TN], F32, tag="z", name=f"zq_{it}_{p}")
                nc.tensor.matmul(zq[:], c["we1b"][:, p, :], hb[:], start=True, stop=True)
                z = p_z.tile([128, TN], BF16, tag="z_sb", bufs=18, name=f"z_{it}_{p}")
                nc.scalar.activation(z[:], zq[:], AF.Gelu, bias=c["eb1"][:, p:p + 1])
                z_sb.append(z)
            z2_sb = []
            for p in range(8):
                z2 = p_z2.tile([128, TN], BF16, tag="z2_sb", bufs=18, name=f"z2_{it}_{p}")
                eng = nc.gpsimd if p < int(os.environ.get('KZ2POOL', '0')) else nc.vector
                eng.tensor_tensor(z2[:], z_sb[p][:], z_sb[p][:], op=ALU.mult)
                z2_sb.append(z2)

            yield  # ==== P7: experts fc2 + LN stats (accumulation groups) ====
            fc = [ps_fc.tile([128, TN], F32, tag="fc", name=f"fc_{it}_{i}")
                  for i in range(2)]
            for grp in range(2):
                for j in range(4):
                    p = 4 * grp + j
                    nc.tensor.matmul(fc[grp][32 * j:32 * j + 32, :],
                                     c["we2b"][:, p, :], z_sb[p][:],
                                     start=True, stop=False, tile_position=(0, 32 * j))
                    nc.tensor.matmul(fc[grp][32 * j:32 * j + 32, :],
                                     c["zwb"], z2_sb[p][:],
                                     start=False, stop=True, tile_position=(0, 32 * j))
            fc2sb = []
            for b in range(2):
                t = p_w.tile([128, TN], BF16, tag="fc2sb", bufs=10, name=f"fc2sb_{it}_{b}")
                nc.scalar.copy(t[:], fc[b][:])
                fc2sb.append(t)

            yield  # ==== P8: stat transposes ====
            statB = p_sc.tile([128, NCH, 2, 4, 2, 2], F32, tag="statB", bufs=3,
                              name=f"statB_{it}")
            for b in range(2):
                psT = ps_sm.tile([128, NCH, 128], BF16, tag="sm", name=f"psT_{it}_{b}")
                for ch in range(NCH):
                    nc.tensor.transpose(psT[:, ch, :],
                                        fc2sb[b][:, 128 * ch:128 * (ch + 1)],
                                        identb)
                psT6 = psT[:].rearrange("p c (j v s q) -> p c j v s q",
                                        j=4, v=8, s=2, q=2)
                nc.vector.tensor_copy(statB[:, :, b], psT6[:, :, :, 0])
            muB = statB[:, :, :, :, 0, :]   # [128, NCH, 2, 4, 2] e-ordered (b,j,q)
            m2B = statB[:, :, :, :, 1, :]

            yield  # ==== P9: pass B (expert LN rs + top-2 gate) ====
            tmpB = p_sc.tile([128, NCH, 2, 4, 2], F32, tag="tmpB", bufs=3, name=f"tmpB_{it}")
            nc.gpsimd.tensor_tensor(tmpB[:], muB, muB, op=ALU.mult)
            vB = p_sc.tile([128, NCH, 2, 4, 2], F32, tag="vB", bufs=3, name=f"vB_{it}")
            nc.gpsimd.scalar_tensor_tensor(vB[:], m2B, EPS_LN, tmpB[:],
                                           op0=ALU.add, op1=ALU.subtract)
            rsB = p_sc.tile([128, NCH, 2, 4, 2], F32, tag="rsB", bufs=3, name=f"rsB_{it}")
            _newton_rsqrt(nc, p_sc, vB[:], rsB[:], [128, NCH, 2, 4, 2],
                          f"nB_{it}", niter=1, eng=nc.gpsimd)

            vm8 = p_sc.tile([128, NCH, 8], F32, tag="vm8", bufs=3, name=f"vm8_{it}")
            for ch in range(NCH):
                nc.vector.max(vm8[:, ch, :], gcp[:, ch, :])
            dg = p_sc.tile([128, NCH], F32, tag="dg", bufs=3, name=f"dg_{it}")
            nc.vector.tensor_tensor(dg[:], vm8[:, :, 0], vm8[:, :, 1], op=ALU.subtract)
            th = p_sc.tile([128, NCH], F32, tag="th", bufs=3, name=f"th_{it}")
            nc.scalar.activation(th[:], dg[:], AF.Tanh, scale=0.5)
            w12 = p_sc.tile([128, NCH, 2], F32, tag="w12", bufs=3, name=f"w12_{it}")
            nc.vector.tensor_scalar(w12[:, :, 0], th[:], 0.5, 0.5, op0=ALU.mult, op1=ALU.add)
            nc.vector.tensor_scalar(w12[:, :, 1], th[:], -0.5, 0.5, op0=ALU.mult, op1=ALU.add)

            is1 = p_sc.tile([128, NCH, E], F32, tag="is1", bufs=3, name=f"is1_{it}")
            nc.vector.tensor_tensor(is1[:], gcp[:], _bc(vm8[:, :, 0:1], E), op=ALU.is_equal)
            is2 = p_sc.tile([128, NCH, E], F32, tag="is2", bufs=3, name=f"is2_{it}")
            nc.vector.tensor_tensor(is2[:], gcp[:], _bc(vm8[:, :, 1:2], E), op=ALU.is_equal)
            w1t = p_sc.tile([128, NCH, E], F32, tag="w1t", bufs=3, name=f"w1t_{it}")
            nc.gpsimd.tensor_tensor(w1t[:], is1[:], _bc(w12[:, :, 0:1], E), op=ALU.mult)
            w2t = p_sc.tile([128, NCH, E], F32, tag="w2t", bufs=3, name=f"w2t_{it}")
            nc.gpsimd.tensor_tensor(w2t[:], is2[:], _bc(w12[:, :, 1:2], E), op=ALU.mult)

            # back block (bf16): cols 0-15 wsm, 16-31 w, 32-47 ws, 48-63 pad
            backB = p_sc.tile([128, NCH, 64], BF16, tag="backB", bufs=4, name=f"backB_{it}")
            nc.gpsimd.memset(backB[:].rearrange("p c k -> p (c k)"), 0.0)
            nc.gpsimd.tensor_tensor(backB[:, :, 16:32], w1t[:], w2t[:], op=ALU.add)
            rsBf = rsB[:].rearrange("p c b j q -> p c (b j q)")
            nc.gpsimd.tensor_tensor(backB[:, :, 32:48], backB[:, :, 16:32], rsBf,
                                    op=ALU.mult)
            ws5 = backB[:, :, 32:48].rearrange("p c (b j q) -> p c b j q",
                                               b=2, j=4, q=2)
            wsm5 = backB[:, :, 0:16].rearrange("p c (b j q) -> p c b j q",
                                               b=2, j=4, q=2)
            nc.gpsimd.tensor_tensor(wsm5, ws5, muB, op=ALU.mult)

            yield  # ==== P10: weight-block transpose ====
            psBB = ps_sm.tile([128, 2, 128], BF16, tag="sm", name=f"psBB_{it}")
            backBv = backB[:].rearrange("p c k -> p (c k)")
            for hh in range(2):
                nc.tensor.transpose(psBB[:, hh, :],
                                    backBv[:, 128 * hh:128 * (hh + 1)], identb)
            cf = p_sc.tile([48, TN], BF16, tag="cf", bufs=4, name=f"cf_{it}")
            cfv = cf[:].rearrange("p (h c q) -> p h c q", h=2, c=2, q=128)
            nc.vector.tensor_copy(cfv[:, :, 0, :], psBB[0:48, :, :])
            nc.vector.tensor_copy(cfv[:, :, 1, :], psBB[64:112, :, :])

            yield  # ==== P11: ws broadcast + weighted fc2 ====
            prods = []
            for b in range(2):
                wsr = ps_sm.tile([128, TN], F32, tag="sm", name=f"wsr_{it}_{b}")
                nc.tensor.matmul(wsr[:], c["wsbb"][32:48, b, :], cf[32:48, :],
                                 start=True, stop=True)
                prod = p_w.tile([128, TN], BF16, tag="prod", bufs=6, name=f"prod_{it}_{b}")
                nc.vector.tensor_tensor(prod[:], fc2sb[b][:], wsr[:], op=ALU.mult)
                prods.append(prod)

            yield  # ==== P12: combine ====
            lg = ps_sm.tile([NCLS, TN], F32, tag="sm", name=f"lg_{it}")
            nc.tensor.matmul(lg[:], c["msumb"], prods[0][:], start=True, stop=False)
            nc.tensor.matmul(lg[:], c["msumb"], prods[1][:], start=False, stop=False)
            nc.tensor.matmul(lg[:], c["gw2cb"], cf[0:32, :], start=False, stop=True)
            lsb = p_out.tile([NCLS, TN], F32, tag="lsb", bufs=4, name=f"lsb_{it}")
            nc.scalar.copy(lsb[:], lg[:])

            yield  # ==== P13: output transpose + DMA ====
            psL = ps_sm.tile([128, NCH * NCLS], F32, tag="sm", name=f"psL_{it}")
            for ch in range(NCH):
                nc.tensor.transpose(psL[:, NCLS * ch:NCLS * (ch + 1)],
                                    lsb[:, 128 * ch:128 * (ch + 1)],
                                    identf[0:NCLS, 0:NCLS])
            osb = p_out.tile([128, NCH, NCLS], F32, tag="osb", bufs=3, name=f"osb_{it}")
            nc.vector.tensor_copy(osb[:], psL[:])
            nc.sync.dma_start(d_out.ap()[it].rearrange("c p k -> p c k"), osb[:])

        # 14-phase software pipeline, oldest tile first within each step so
        # slot-freeing work always precedes the allocations that reuse slots,
        # and every cross-engine dependency crosses a step boundary.
        NPH = 14
        gens = {}
        for k in range(ntiles + NPH - 1):
            if k < ntiles:
                gens[k] = tile_body(k)
            for idx in sorted(gens):
                if next(gens[idx], StopIteration) is StopIteration:
                    del gens[idx]

    nc.compile()
    return nc


def _newton_rsqrt(nc, pool, v_ap, out_ap, shape, tag, niter=2, eng=None):
    """out = 1/sqrt(v) via quake seed + Newton iterations."""
    eng = eng or nc.vector
    r = pool.tile(shape, F32, tag=tag[:3] + "_r", name=tag + "_r")
    t = pool.tile(shape, F32, tag=tag[:3] + "_t", name=tag + "_t")
    eng.tensor_scalar(r[:].bitcast(I32), v_ap.bitcast(I32), 1, None,
                      op0=ALU.logical_shift_right)
    eng.tensor_scalar(r[:].bitcast(I32), r[:].bitcast(I32), -1, 0x5F3759DF,
                      op0=ALU.mult, op1=ALU.add)
    for i in range(niter):
        dst = out_ap if i == niter - 1 else r[:]
        eng.tensor_tensor(t[:], r[:], r[:], op=ALU.mult)
        eng.scalar_tensor_tensor(t[:], t[:], -0.5, v_ap, op0=ALU.mult, op1=ALU.mult)
        eng.scalar_tensor_tensor(dst, t[:], 1.5, r[:], op0=ALU.add, op1=ALU.mult)


# ---------------------------------------------------------------------------
# host-side weight prep
# ---------------------------------------------------------------------------
def prep_consts(inp):
    f = np.float32
    import ml_dtypes
    bf = ml_dtypes.bfloat16
    gU = np.asarray(inp["gU"], np.float64)
    gb = np.asarray(inp["gb"], np.float64)
    e_w1, e_b1 = np.asarray(inp["e_w1"], f), np.asarray(inp["e_b1"], f)
    e_g, e_beta = np.asarray(inp["e_g"], f), np.asarray(inp["e_beta"], f)
    e_w2, e_b2 = np.asarray(inp["e_w2"], f), np.asarray(inp["e_b2"], f)
    bb_g = np.asarray(inp["bb_g"], np.float64)
    bb_beta = np.asarray(inp["bb_beta"], np.float64)

    vals32 = {}
    vals32["identf"] = np.eye(128, dtype=f)
    vals32["wbb1"] = np.asarray(inp["bb_w1"], f)
    vals32["wbb2"] = np.asarray(inp["bb_w2"], f)
    vals32["b1"] = np.asarray(inp["bb_b1"], f).reshape(EMB, 1)
    vals32["b2"] = np.asarray(inp["bb_b2"], f).reshape(EMB, 1)
    vals32["beta"] = bb_beta.astype(f).reshape(EMB, 1)
    st = np.zeros((128, 2), f)
    st[0:64, 0] = 1.0 / 64
    st[64:128, 1] = 1.0 / 64
    vals32["stat2"] = st
    wgU0 = np.zeros((EMB, 128), np.float64)
    for e in range(E):
        wgU0[:, e * RANK:(e + 1) * RANK] = gU[e] * bb_g[:, None]
    vals32["wgU0"] = wgU0.astype(f)
    _wgU0_f64 = wgU0
    gs = np.zeros((128, E), f)
    for e in range(E):
        gs[e * RANK:(e + 1) * RANK, e] = 1.0
    vals32["gsum"] = gs
    eb1 = np.zeros((128, 8), f)
    for p in range(8):
        eb1[0:64, p] = e_b1[2 * p]
        eb1[64:128, p] = e_b1[2 * p + 1]
    vals32["eb1"] = eb1

    vals16 = {}
    vals16["identb"] = np.eye(128, dtype=f)
    stl = np.zeros((2, 128), np.float64)
    stl[0, 0:64] = bb_g
    stl[1, 64:128] = -bb_g
    vals16["stlb"] = stl
    we1 = np.zeros((EMB, 8, 128), f)
    for p in range(8):
        we1[:, p, 0:64] = e_w1[2 * p]
        we1[:, p, 64:128] = e_w1[2 * p + 1]
    vals16["we1b"] = we1.reshape(EMB, 1024)
    # fc2 lhsT: cols 0/1 mu weights, 2/3 zero (m2 via zwb), 4:14 cls e0, 14:24 cls e1
    we2 = np.zeros((128, 8, 32), f)
    for p in range(8):
        e0, e1 = 2 * p, 2 * p + 1
        we2[0:64, p, 0] = 1.0 / 64
        we2[64:128, p, 1] = 1.0 / 64
        we2[0:64, p, 4:14] = e_g[e0][:, None] * e_w2[e0]
        we2[64:128, p, 14:24] = e_g[e1][:, None] * e_w2[e1]
    vals16["we2b"] = we2.reshape(128, 256)
    zw = np.zeros((128, 32), f)
    zw[0:64, 2] = 1.0 / 64
    zw[64:128, 3] = 1.0 / 64
    vals16["zwb"] = zw
    wsb = np.zeros((48, 2, 128), f)
    for e in range(E):
        b, j, q = e // 8, (e % 8) // 2, e % 2
        wsb[32 + e, b, 32 * j + 4 + 10 * q:32 * j + 14 + 10 * q] = 1.0
    vals16["wsbb"] = wsb.reshape(48, 256)
    ms = np.zeros((128, NCLS), f)
    for j in range(4):
        for q in range(2):
            for cc in range(NCLS):
                ms[32 * j + 4 + 10 * q + cc, cc] = 1.0
    vals16["msumb"] = ms
    gw2 = np.einsum("ed,edc->ec", e_g, e_w2)
    cst = np.einsum("ed,edc->ec", e_beta, e_w2) + e_b2
    gw2c = np.zeros((2 * E, NCLS), f)
    gw2c[0:E] = -gw2
    gw2c[E:2 * E] = cst
    vals16["gw2cb"] = gw2c
    # stat gather: row 32j+2s+q -> col 4j+2s+q
    pg = np.zeros((128, 32), f)
    for j in range(4):
        for s in range(2):
            for q in range(2):
                pg[32 * j + 2 * s + q, 4 * j + 2 * s + q] = 1.0
    vals16["permg"] = pg

    w1 = np.asarray(inp["bb_w1"], np.float64)
    w1h = w1.astype(np.float16)
    w1l = (w1 - w1h.astype(np.float64)).astype(np.float16)
    ch16 = np.zeros((128, 388), np.float16)
    ch16[0:IN_F, 0:EMB] = w1h
    ch16[0:IN_F, EMB:2 * EMB] = w1l
    u0 = _wgU0_f64
    u0h = u0.astype(np.float16)
    u0l = (u0 - u0h.astype(np.float64)).astype(np.float16)
    ch16[0:EMB, 128:256] = u0h
    ch16[0:EMB, 256:384] = u0l
    ch16[0:64, 384] = np.float16(1.0 / 64)
    ch16[64:128, 385] = np.float16(1.0 / 64)

    cf32 = np.zeros((128, CF32_COLS), f)
    for name, (p, o, w) in CF32_OFF.items():
        cf32[0:p, o:o + w] = vals32[name]
    cb16 = np.zeros((128, CB16_COLS), bf)
    for name, (p, o, w) in CB16_OFF.items():
        cb16[0:p, o:o + w] = np.asarray(vals16[name], np.float64).astype(bf)
    return {"cf32": cf32, "cb16": cb16, "ch16": ch16}


def prep_user_tables(inp):
    """uV gather table [NUSERS,128] plus per-user gate tables B, D [NUSERS,E].
    All in float64 then rounded once to f32."""
    gU = np.asarray(inp["gU"], np.float64)
    gV = np.asarray(inp["gV"], np.float64)
    gb = np.asarray(inp["gb"], np.float64)
    ut = np.asarray(inp["ut"], np.float64)
    bb_g = np.asarray(inp["bb_g"], np.float64)
    bb_beta = np.asarray(inp["bb_beta"], np.float64)
    wgU = np.zeros((EMB, 128), np.float64)
    for e in range(E):
        wgU[:, e * RANK:(e + 1) * RANK] = gU[e]
    uV = np.einsum("ud,edr->uer", ut, gV).reshape(NUSERS, 128)  # [u, e*8+r]
    cg = (bb_g @ wgU).reshape(E, RANK)       # wgU^T g
    cb = (bb_beta @ wgU).reshape(E, RANK)    # wgU^T beta
    uV3 = uV.reshape(NUSERS, E, RANK)
    Btab = np.einsum("er,uer->ue", cg, uV3)
    Dtab = np.einsum("er,uer->ue", cb, uV3) + gb[None, :]
    return uV.astype(np.float32), Btab.astype(np.float32), Dtab.astype(np.float32)


def shard_inputs(x, user_ids, inp, b_core):
    """x [B,80] -> per-core [nt,80,512] feature-major; uV gathered+transposed;
    B/D tables gathered batch-major."""
    ncores = x.shape[0] // b_core
    nt = b_core // TN
    xr = x.astype(np.float64)
    xh = xr.astype(np.float16)
    xl = (xr - xh.astype(np.float64)).astype(np.float16)
    xhs = xh.reshape(ncores, nt, TN, IN_F).transpose(0, 1, 3, 2)
    xls = xl.reshape(ncores, nt, TN, IN_F).transpose(0, 1, 3, 2)
    xs = np.ascontiguousarray(np.concatenate([xhs, xls], axis=3))  # [.., 80, 1024]
    uV, Btab, Dtab = prep_user_tables(inp)
    u = uV[user_ids]                                   # [B, 128]
    us = np.ascontiguousarray(
        u.reshape(ncores, nt, TN, 128).transpose(0, 1, 3, 2))
    # batch-major: sample s at (row=s%128, ch=s//128); B and D side by side
    BD = np.concatenate([Btab[user_ids], Dtab[user_ids]], axis=-1)  # [B, 2E]
    BDg = BD.reshape(ncores, nt, NCH, 128, 2 * E)
    BDt = np.ascontiguousarray(BDg.transpose(0, 1, 3, 2, 4))  # [.., 128, NCH, 2E]
    return xs, us, BDt


_CACHE = {}


def _get_program(b_core, mmdt="hybrid"):
    key = (b_core, mmdt)
    if key not in _CACHE:
        _CACHE[key] = build_program(b_core, mmdt)
    return _CACHE[key]


def build_in_maps(inputs):
    x = np.asarray(inputs["x"], np.float64).reshape(B, IN_F)
    uids = np.asarray(inputs["user_ids"]).astype(np.int64)
    cns = prep_consts({k: np.asarray(v) for k, v in inputs.items()})
    xs, us, BDt = shard_inputs(x, uids, inputs, B_CORE)
    in_maps = []
    for k in range(NCORES):
        m = dict(cns)
        m["x"] = xs[k]
        m["u"] = us[k]
        m["BD"] = BDt[k]
        in_maps.append(m)
    return in_maps


def kernel(**inputs):
    from concourse.bass_utils import run_bass_kernel_spmd
    nc = _get_program(B_CORE)
    in_maps = build_in_maps(inputs)
    res = run_bass_kernel_spmd(nc, in_maps, core_ids=list(range(NCORES)))
    out = np.concatenate([r["out"].reshape(B_CORE, NCLS) for r in res.results], axis=0)
    return out.astype(np.float32)
